# revision 10
# baseline (speedup 1.0000x reference)
"""Deformable Conv2d (v2, torchvision semantics) Trainium2 Bass kernel.

Problem: nn_DeformableConv2d_76321568850098
  x (4,256,64,64) f32; main weight (256,256,3,3); offset conv (18 ch) and
  mask conv (9 ch) computed from x; bilinear sampling at learned offsets;
  out (4,256,64,64) f32.

Sharding: 8 cores = 4 batches x 2 pixel-halves (rows 0-31 / 32-63).
Each core gets a 44-row zero-padded slab of its batch image and computes
out[b, :, half] for its 2048 pixels.

Per-core on-device pipeline:
  1. load slab (256, 2816) f32, cast bf16
  2. offset/mask conv: im2col (shifted views, edge-masked) x 18 ck-block
     matmuls -> (27, 2048) maps
  3. PE-transpose maps to pixel-on-partition layout; compute floor/frac/
     validity/bilinear corner weights (DVE) and int16 gather indices
  4. PE-transpose slab to (token, channel) bf16 in DRAM scratch
  5. dma_gather (2 row-corners x 9 taps, elem = 2 pixels x 256ch bf16)
  6. corner-combine + transpose to (ck, pix) via diagonal-matrix matmuls
     accumulated in PSUM (bilinear weights folded into the diagonals)
  7. main matmul W^T(2304x256) @ samp(2304x2048), bias, DMA out
"""
import os
import numpy as np
import ml_dtypes
from contextlib import ExitStack

import concourse.bass as bass
import concourse.tile as tile
import concourse.bacc as bacc
from concourse import mybir
from concourse.masks import make_identity

AF = mybir.ActivationFunctionType
OP = mybir.AluOpType
bf16 = ml_dtypes.bfloat16

# problem constants
B, C, O, H, W = 4, 256, 256, 64, 64
K, KK = 3, 9
NPIX_HALF = 2048          # pixels per core
NSLOT = 16                # 128-pixel slots per core
PAD_ROWS = 6
SLAB_ROWS = 32 + 2 * PAD_ROWS          # 44
NTOK = SLAB_ROWS * W                   # 2816
OWN0 = PAD_ROWS * W                    # 384: first own pixel within slab
CB = 2                    # 128-channel blocks
TK = KK * CB              # 18 contraction blocks
N_CORES = 8


def _ap(t, offset_elems, dims):
    """Manual AP on a tile: dims = [[stride, n], ...] in elements."""
    return bass.AP(tensor=t.tensor, offset=t.offset + offset_elems, ap=dims)


def build_program(nc, debug_outputs=False):
    dt = mybir.dt
    # ---------------- DRAM I/O ----------------
    x_slab = nc.dram_tensor("x_slab", [C, NTOK], dt.float32, kind="ExternalInput")
    wmain_d = nc.dram_tensor("wmain", [TK, 128, O], dt.bfloat16, kind="ExternalInput")
    woff_d = nc.dram_tensor("woff", [TK, 128, 32], dt.bfloat16, kind="ExternalInput")
    bias_d = nc.dram_tensor("bias_o", [128, 2], dt.float32, kind="ExternalInput")
    bcat_d = nc.dram_tensor("bcat27", [32, 1], dt.float32, kind="ExternalInput")
    cy_d = nc.dram_tensor("cy_tab", [128, NSLOT, KK], dt.float32, kind="ExternalInput")
    cx_d = nc.dram_tensor("cx_tab", [128, NSLOT, KK], dt.float32, kind="ExternalInput")
    gb_d = nc.dram_tensor("gbase", [128, 1], dt.float32, kind="ExternalInput")
    vyc_d = nc.dram_tensor("vyc", [128, 2], dt.float32, kind="ExternalInput")
    out_d = nc.dram_tensor("out", [O, NPIX_HALF], dt.float32, kind="ExternalOutput")
    dbg = {}
    if debug_outputs:
        dbg["off"] = nc.dram_tensor("dbg_off", [32, NPIX_HALF], dt.float32, kind="ExternalOutput")
        dbg["cw"] = nc.dram_tensor("dbg_cw", [128, KK, 64], dt.float32, kind="ExternalOutput")
        dbg["g00"] = nc.dram_tensor("dbg_g00", [128, KK, NSLOT], dt.int32, kind="ExternalOutput")
        dbg["idx"] = nc.dram_tensor("dbg_idx", [128, 2, KK, 128], dt.int16, kind="ExternalOutput")
        dbg["samp"] = nc.dram_tensor("dbg_samp", [128, TK, NPIX_HALF], dt.bfloat16, kind="ExternalOutput")

    with tile.TileContext(nc) as tc, ExitStack() as ctx:
        consts = ctx.enter_context(tc.tile_pool(name="consts", bufs=1))
        xf_pool = ctx.enter_context(tc.tile_pool(name="xf", bufs=1))
        xpad_pool = ctx.enter_context(tc.tile_pool(name="xpad", bufs=1))
        xs_pool = ctx.enter_context(tc.tile_pool(name="xs", bufs=3))
        ph2 = ctx.enter_context(tc.tile_pool(name="ph2", bufs=1))
        dbgp = ctx.enter_context(tc.tile_pool(name="dbgp", bufs=1))
        offp = ctx.enter_context(tc.tile_pool(name="offp", bufs=1))
        idxp = ctx.enter_context(tc.tile_pool(name="idxp", bufs=1))
        gath_pool = ctx.enter_context(tc.tile_pool(name="gath", bufs=4))
        dpool = ctx.enter_context(tc.tile_pool(name="dpool", bufs=2))
        samp_pool = ctx.enter_context(tc.tile_pool(name="samp", bufs=2))
        xt_pool = ctx.enter_context(tc.tile_pool(name="xt", bufs=1))
        outp = ctx.enter_context(tc.tile_pool(name="outp", bufs=3))
        psA = ctx.enter_context(tc.tile_pool(name="psA", bufs=2, space="PSUM"))
        psO = ctx.enter_context(tc.tile_pool(name="psO", bufs=4, space="PSUM"))
        psT = ctx.enter_context(tc.tile_pool(name="psT", bufs=2, space="PSUM"))
        dram_scratch = ctx.enter_context(tc.tile_pool(name="drs", bufs=1, space="DRAM"))

        # ---------------- constants ----------------
        ident_bf = consts.tile([128, 128], dt.bfloat16)
        make_identity(nc, ident_bf[:])
        ident32 = consts.tile([32, 32], dt.float32)
        make_identity(nc, ident32[:])
        wmain_sb = consts.tile([128, TK, O], dt.bfloat16)
        nc.sync.dma_start(out=wmain_sb[:], in_=bass.AP(
            tensor=wmain_d, offset=0, ap=[[O, 128], [128 * O, TK], [1, O]]))
        woff_sb = consts.tile([128, TK, 32], dt.bfloat16)
        nc.sync.dma_start(out=woff_sb[:], in_=bass.AP(
            tensor=woff_d, offset=0, ap=[[32, 128], [128 * 32, TK], [1, 32]]))
        bias_sb = consts.tile([128, 2], dt.float32)
        nc.sync.dma_start(out=bias_sb[:], in_=bias_d.ap())
        bcat_sb = consts.tile([32, 1], dt.float32)
        nc.sync.dma_start(out=bcat_sb[:], in_=bcat_d.ap())
        cy_sb = consts.tile([128, NSLOT, KK], dt.float32)
        nc.sync.dma_start(out=cy_sb[:], in_=cy_d.ap())
        cx_sb = consts.tile([128, NSLOT, KK], dt.float32)
        nc.sync.dma_start(out=cx_sb[:], in_=cx_d.ap())
        gb_sb = consts.tile([128, 1], dt.float32)
        nc.sync.dma_start(out=gb_sb[:], in_=gb_d.ap())
        vyc_sb = consts.tile([128, 2], dt.float32)
        nc.sync.dma_start(out=vyc_sb[:], in_=vyc_d.ap())

        # ---------------- load slab + cast bf16 ----------------
        xpad = xpad_pool.tile([128, CB, NTOK], dt.bfloat16)
        for cb in range(CB):
            xf = xf_pool.tile([128, NTOK], dt.float32)
            nc.sync.dma_start(out=xf[:], in_=bass.AP(
                tensor=x_slab, offset=cb * 128 * NTOK, ap=[[NTOK, 128], [1, NTOK]]))
            nc.vector.tensor_copy(xpad[:, cb, :], xf[:])

        # ---------------- xT staging: PE transpose -> SBUF -> DRAM ----------
        xT_dram = dram_scratch.tile([NTOK, C], dt.bfloat16)
        xt_sb = xt_pool.tile([128, SLAB_ROWS // 2, CB, 128], dt.bfloat16)
        for ch in range(SLAB_ROWS // 2):      # 22 chunks of 128 tokens
            for cb in range(CB):
                pt = psT.tile([128, 128], dt.bfloat16)
                nc.tensor.transpose(pt[:], xpad[:, cb, ch * 128:(ch + 1) * 128],
                                    ident_bf[:])
                eng = nc.scalar if (ch + cb) % 2 == 0 else nc.vector
                if eng is nc.scalar:
                    nc.scalar.copy(xt_sb[:, ch, cb, :], pt[:])
                else:
                    nc.vector.tensor_copy(xt_sb[:, ch, cb, :], pt[:])
        # one DMA: (lane, ch, cb, c) -> dram token-major
        nc.sync.dma_start(
            out=bass.AP(tensor=xT_dram.tensor, offset=xT_dram.offset,
                        ap=[[C, 128], [128 * C, SLAB_ROWS // 2], [128, CB], [1, 128]]),
            in_=xt_sb[:])

        # ---------------- offset/mask conv ----------------
        off_ps = [psO.tile([32, 512], dt.float32, tag="po", name=f"off_ps{i}")
                  for i in range(4)]
        for t in range(TK):
            k, cb = t // CB, t % CB
            ky, kx = k // K, k % K
            dk = (ky - 1) * W + (kx - 1)
            xs = xs_pool.tile([128, NPIX_HALF], dt.bfloat16, tag="xs")
            nc.scalar.copy(xs[:], xpad[:, cb, OWN0 + dk:OWN0 + dk + NPIX_HALF])
            if kx == 0:
                nc.vector.memset(_ap(xs, 0, [xs.ap[0], [W, NPIX_HALF // W]]), 0.0)
            elif kx == 2:
                nc.vector.memset(_ap(xs, W - 1, [xs.ap[0], [W, NPIX_HALF // W]]), 0.0)
            for nb in range(4):
                nc.tensor.matmul(off_ps[nb][:], woff_sb[:, t, :],
                                 xs[:, nb * 512:(nb + 1) * 512],
                                 start=(t == 0), stop=(t == TK - 1))
        off_sb = offp.tile([32, NPIX_HALF], dt.float32)
        for nb in range(4):
            nc.scalar.activation(off_sb[:, nb * 512:(nb + 1) * 512], off_ps[nb][:],
                                 AF.Identity, bias=bcat_sb[:])
        if debug_outputs:
            nc.sync.dma_start(out=dbg["off"].ap(), in_=off_sb[:])

        # ---------------- transpose offsets to pixel-major ----------------
        offT_ps = psA.tile([128, 512], dt.float32, tag="ps")
        for slot in range(NSLOT):
            nc.tensor.transpose(offT_ps[:, slot * 32:(slot + 1) * 32],
                                off_sb[:, slot * 128:(slot + 1) * 128], ident32[:])
        offT = ph2.tile([128, NSLOT, 32], dt.float32, tag="offT")
        nc.scalar.copy(offT[:], offT_ps[:])

        # ---------------- phase 2: weights + indices ----------------
        F = NSLOT * KK  # 144
        def pt_tile(tag):
            return ph2.tile([128, NSLOT, KK], dt.float32, tag=tag, name=tag)

        dy_ap = _ap(offT, 0, [offT.ap[0], [32, NSLOT], [2, KK]])
        dx_ap = _ap(offT, 1, [offT.ap[0], [32, NSLOT], [2, KK]])
        ml_ap = _ap(offT, 18, [offT.ap[0], [32, NSLOT], [1, KK]])

        pyt = pt_tile("pyt")
        nc.vector.tensor_tensor(pyt[:], dy_ap, cy_sb[:], op=OP.add)
        pxt = pt_tile("pxt")
        nc.vector.tensor_tensor(pxt[:], dx_ap, cx_sb[:], op=OP.add)
        mt = pt_tile("mt")
        nc.scalar.activation(mt[:], ml_ap, AF.Sigmoid)

        fyi = ph2.tile([128, NSLOT, KK], dt.int32, tag="fyi")
        nc.vector.tensor_scalar_add(fyi[:], pyt[:], 16.0)
        fyf = pt_tile("fyf")
        nc.vector.tensor_scalar_add(fyf[:], fyi[:], -16.0)
        fxi = ph2.tile([128, NSLOT, KK], dt.int32, tag="fxi")
        nc.vector.tensor_scalar_add(fxi[:], pxt[:], 16.0)
        fxf = pt_tile("fxf")
        nc.vector.tensor_scalar_add(fxf[:], fxi[:], -16.0)

        wy1 = pt_tile("wy1")
        nc.vector.tensor_tensor(wy1[:], pyt[:], fyf[:], op=OP.subtract)
        wy0 = pt_tile("wy0")
        nc.vector.tensor_scalar(wy0[:], wy1[:], -1.0, 1.0, op0=OP.mult, op1=OP.add)
        wx1 = pt_tile("wx1")
        nc.vector.tensor_tensor(wx1[:], pxt[:], fxf[:], op=OP.subtract)
        wx0 = pt_tile("wx0")
        nc.vector.tensor_scalar(wx0[:], wx1[:], -1.0, 1.0, op0=OP.mult, op1=OP.add)

        def win_check(src, center, halfw, tag):
            t1 = pt_tile(tag + "_t")
            nc.vector.tensor_scalar(t1[:], src[:], center, 0.0,
                                    op0=OP.subtract, op1=OP.abs_max)
            v = pt_tile(tag)
            nc.vector.tensor_scalar(v[:], t1[:], halfw, None, op0=OP.is_le)
            return v

        va0 = win_check(fyf, 31.5, 31.9, "va0")      # 0 <= fy <= 63
        va1 = win_check(fyf, 30.5, 31.9, "va1")      # 0 <= fy+1 <= 63
        vx0 = win_check(fxf, 31.5, 31.9, "vx0")
        vx1 = win_check(fxf, 30.5, 31.9, "vx1")
        # slab-range insurance: fy_loc in [-6,36] / fy+1_loc in [-6,36]
        vs0 = win_check(fyf, vyc_sb[:, 0:1], 21.9, "vs0")
        vs1 = win_check(fyf, vyc_sb[:, 1:2], 21.9, "vs1")

        wxv0 = pt_tile("wxv0")
        nc.vector.tensor_tensor(wxv0[:], wx0[:], vx0[:], op=OP.mult)
        wxv1 = pt_tile("wxv1")
        nc.vector.tensor_tensor(wxv1[:], wx1[:], vx1[:], op=OP.mult)
        m0 = pt_tile("m0")
        nc.vector.tensor_tensor(m0[:], mt[:], wy0[:], op=OP.mult)
        nc.vector.tensor_tensor(m0[:], m0[:], va0[:], op=OP.mult)
        nc.vector.tensor_tensor(m0[:], m0[:], vs0[:], op=OP.mult)
        m1 = pt_tile("m1")
        nc.vector.tensor_tensor(m1[:], mt[:], wy1[:], op=OP.mult)
        nc.vector.tensor_tensor(m1[:], m1[:], va1[:], op=OP.mult)
        nc.vector.tensor_tensor(m1[:], m1[:], vs1[:], op=OP.mult)

        # corner weights -> cw_pack [128, KK, 64] bf16 (k, slot*4+corner)
        cw_pack = idxp.tile([128, KK, 64], dt.bfloat16)
        for j, (a, b_) in enumerate(((m0, wxv0), (m0, wxv1), (m1, wxv0), (m1, wxv1))):
            dst = _ap(cw_pack, j, [cw_pack.ap[0], [4, NSLOT], [64, KK]])
            nc.vector.tensor_tensor(dst, a[:], b_[:], op=OP.mult)

        # gather indices
        gt0 = pt_tile("gt0")
        nc.vector.tensor_scalar(gt0[:], fyf[:], 64.0, gb_sb[:], op0=OP.mult, op1=OP.add)
        g00f = pt_tile("g00f")
        nc.vector.tensor_tensor(g00f[:], gt0[:], fxf[:], op=OP.add)
        gmax = float(NTOK - 2)
        gi0 = idxp.tile([128, KK, NSLOT], dt.int16)
        nc.vector.tensor_scalar(
            _ap(gi0, 0, [gi0.ap[0], [1, NSLOT], [NSLOT, KK]]),
            g00f[:], 0.0, gmax, op0=OP.max, op1=OP.min)
        g10t = pt_tile("g10t")
        nc.vector.tensor_scalar(g10t[:], g00f[:], 64.0, 0.0, op0=OP.add, op1=OP.max)
        gi1 = idxp.tile([128, KK, NSLOT], dt.int16)
        nc.vector.tensor_scalar(
            _ap(gi1, 0, [gi1.ap[0], [1, NSLOT], [NSLOT, KK]]),
            g10t[:], gmax, None, op0=OP.min)

        if debug_outputs:
            g00dbg = dbgp.tile([128, KK, NSLOT], dt.int32, tag="gdbg")
            nc.vector.tensor_copy(g00dbg[:], gi0[:])
            nc.sync.dma_start(out=dbg["g00"].ap(), in_=g00dbg[:])
            cwdbg = dbgp.tile([128, KK, 64], dt.float32, tag="cwdbg")
            nc.vector.tensor_copy(cwdbg[:], cw_pack[:])
            nc.sync.dma_start(out=dbg["cw"].ap(), in_=cwdbg[:])

        # fold indices into dma_gather layout: idx_sb[lane<16, r, k, j=slot*8+grp]
        idx_sb = idxp.tile([128, 2, KK, 128], dt.int16)
        for grp in range(8):
            for r, gi in enumerate((gi0, gi1)):
                src = gi[grp * 16:(grp + 1) * 16, :, :]
                dst = _ap(idx_sb, r * KK * 128 + grp,
                          [[idx_sb.ap[0][0], 16], [128, KK], [8, NSLOT]])
                nc.sync.dma_start(out=dst, in_=src)
        # replicate partitions 0-15 -> 16..127 (7 DMAs)
        for rep in range(1, 8):
            nc.sync.dma_start(out=idx_sb[rep * 16:(rep + 1) * 16, :, :, :],
                              in_=idx_sb[0:16, :, :, :])

        if debug_outputs:
            nc.sync.dma_start(out=dbg["idx"].ap(), in_=idx_sb[:])

        # ------- gathers + diag combine + fused main matmul (per half) -------
        xTsrc = bass.AP(tensor=xT_dram.tensor, offset=xT_dram.offset,
                        ap=[[C, NTOK - 1], [1, 2 * C]])
        for hf in range(2):
            out_ps = [psO.tile([128, 512], dt.float32, tag="po",
                               name=f"out_ps{hf}_{i}") for i in range(4)]
            for k in range(KK):
                gts = []
                for r in range(2):
                    gt = gath_pool.tile([128, 8, 2 * C], dt.bfloat16, tag="gt")
                    nc.gpsimd.dma_gather(
                        out_ap=gt[:], in_ap=xTsrc,
                        idxs_ap=idx_sb[:, r, k, hf * 64:(hf + 1) * 64],
                        num_idxs=1024, num_idxs_reg=1024,
                        elem_size=2 * C, elem_step=C, transpose=False)
                    gts.append(gt)
                # diag matrices for this (k, half): [128, 32, 128]
                dmat = dpool.tile([128, 32, 128], dt.bfloat16, tag="dmat")
                in0 = bass.AP(tensor=ident_bf.tensor, offset=ident_bf.offset,
                              ap=[ident_bf.ap[0], [0, 32], [1, 128]])
                in1 = _ap(cw_pack, k * 64 + hf * 32,
                          [cw_pack.ap[0], [1, 32], [0, 128]])
                nc.vector.tensor_tensor(dmat[:], in0, in1, op=OP.mult)
                samp_k = samp_pool.tile([128, CB, 1024], dt.bfloat16, tag="sk")
                for cb in range(CB):
                    for q4 in range(2):
                        sp = psA.tile([128, 512], dt.float32, tag="ps")
                        for s8 in range(q4 * 4, q4 * 4 + 4):
                            for j in range(4):
                                r, s = j // 2, j % 2
                                lhsT = gts[r][:, s8, s * C + cb * 128:
                                              s * C + cb * 128 + 128]
                                rhs = dmat[:, s8 * 4 + j, :]
                                nc.tensor.matmul(
                                    sp[:, (s8 - q4 * 4) * 128:(s8 - q4 * 4 + 1) * 128],
                                    lhsT, rhs, start=(j == 0), stop=(j == 3))
                        dst = samp_k[:, cb, q4 * 512:(q4 + 1) * 512]
                        if (cb + q4) % 2 == 0:
                            nc.scalar.copy(dst, sp[:])
                        else:
                            nc.vector.tensor_copy(dst, sp[:])
                if debug_outputs:
                    for cb in range(CB):
                        nc.sync.dma_start(
                            out=bass.AP(tensor=dbg["samp"], offset=(k * CB + cb) * NPIX_HALF + hf * 1024,
                                        ap=[[TK * NPIX_HALF, 128], [1, 1024]]),
                            in_=samp_k[:, cb, :])
                # fused main matmul: accumulate this k into the out psum tiles
                for cb in range(CB):
                    t = k * CB + cb
                    for ob in range(2):
                        for nb2 in range(2):
                            nc.tensor.matmul(
                                out_ps[ob * 2 + nb2][:],
                                wmain_sb[:, t, ob * 128:(ob + 1) * 128],
                                samp_k[:, cb, nb2 * 512:(nb2 + 1) * 512],
                                start=(t == 0), stop=(t == TK - 1))
            for ob in range(2):
                for nb2 in range(2):
                    ot = outp.tile([128, 512], dt.float32, tag="ot")
                    nc.scalar.activation(ot[:], out_ps[ob * 2 + nb2][:], AF.Identity,
                                         bias=bias_sb[:, ob:ob + 1])
                    nc.sync.dma_start(
                        out=bass.AP(tensor=out_d,
                                    offset=ob * 128 * NPIX_HALF + hf * 1024 + nb2 * 512,
                                    ap=[[NPIX_HALF, 128], [1, 512]]),
                        in_=ot[:])
    return nc


# ------------------------ host side ------------------------

def pack_inputs(x, weight, bias, off_w, off_b, mask_w, mask_b):
    """Build the 8 per-core input maps."""
    x = np.asarray(x, np.float32)
    weight = np.asarray(weight, np.float32)
    bias = np.asarray(bias, np.float32)
    wcat = np.concatenate([np.asarray(off_w, np.float32),
                           np.asarray(mask_w, np.float32)], 0)   # (27,256,3,3)
    bcat = np.concatenate([np.asarray(off_b, np.float32),
                           np.asarray(mask_b, np.float32)], 0)   # (27,)

    wmain = np.zeros((TK, 128, O), bf16)
    woff = np.zeros((TK, 128, 32), bf16)
    for k in range(KK):
        ky, kx = k // K, k % K
        for cb in range(CB):
            t = k * CB + cb
            wmain[t] = weight[:, cb * 128:(cb + 1) * 128, ky, kx].T.astype(bf16)
            woff[t, :, :27] = wcat[:, cb * 128:(cb + 1) * 128, ky, kx].T.astype(bf16)
    bias_o = bias.reshape(2, 128).T.copy()               # [128, 2]
    bcat27 = np.zeros((32, 1), np.float32)
    bcat27[:27, 0] = bcat

    lane = np.arange(128)
    slot = np.arange(NSLOT)
    p_loc = slot[None, :] * 128 + lane[:, None]          # [128, 16]
    h_loc = (p_loc // W).astype(np.float32)
    w_loc = (p_loc % W).astype(np.float32)
    ky_t = (np.arange(KK) // K).astype(np.float32)
    kx_t = (np.arange(KK) % K).astype(np.float32)

    in_maps = []
    for core in range(N_CORES):
        b, half = core // 2, core % 2
        h0 = half * 32
        # zero-padded slab rows [h0-6, h0+38)
        slab = np.zeros((C, SLAB_ROWS, W), np.float32)
        lo, hi = h0 - PAD_ROWS, h0 + 32 + PAD_ROWS
        slo, shi = max(0, lo), min(H, hi)
        slab[:, slo - lo:shi - lo, :] = x[b, :, slo:shi, :]
        cy = (h0 + h_loc)[:, :, None] + ky_t[None, None, :] - 1.0
        cx = w_loc[:, :, None] + kx_t[None, None, :] - 1.0
        gbase = np.full((128, 1), (PAD_ROWS - h0) * W, np.float32)
        vyc = np.zeros((128, 2), np.float32)
        vyc[:, 0] = h0 + 15.0
        vyc[:, 1] = h0 + 14.5
        in_maps.append({
            "x_slab": np.ascontiguousarray(slab.reshape(C, NTOK)),
            "wmain": wmain, "woff": woff,
            "bias_o": np.ascontiguousarray(bias_o), "bcat27": bcat27,
            "cy_tab": np.ascontiguousarray(cy.astype(np.float32)),
            "cx_tab": np.ascontiguousarray(cx.astype(np.float32)),
            "gbase": gbase, "vyc": vyc,
        })
    return in_maps


_CACHED = {}


def _get_program():
    if "nc" not in _CACHED:
        nc = bacc.Bacc("TRN2", target_bir_lowering=False, debug=False,
                       num_devices=N_CORES)
        build_program(nc)
        nc.compile()
        _CACHED["nc"] = nc
    return _CACHED["nc"]


def run_traced(inputs, trace=False, trace_cores=None):
    """Run on HW; returns (out, BassKernelResults)."""
    from concourse.bass_utils import run_bass_kernel_spmd
    nc = _get_program()
    in_maps = pack_inputs(**inputs)
    res = run_bass_kernel_spmd(nc, in_maps, core_ids=list(range(N_CORES)),
                               trace=trace, trace_cores=trace_cores)
    out = np.zeros((B, O, H, W), np.float32)
    for core in range(N_CORES):
        b, half = core // 2, core % 2
        o = np.asarray(res.results[core]["out"]).reshape(O, 32, W)
        out[b, :, half * 32:(half + 1) * 32, :] = o
    return out, res


def kernel(x, weight, bias, off_w, off_b, mask_w, mask_b):
    out, _ = run_traced(dict(x=x, weight=weight, bias=bias, off_w=off_w,
                             off_b=off_b, mask_w=mask_w, mask_b=mask_b))
    return out
